# revision 39
# baseline (speedup 1.0000x reference)
"""GQA forward (B=2,T=2048,E=2048,H=32,HKV=8,D=64, RoPE, causal) on 8 trn2 cores.

Sharding: tensor-parallel over kv-heads. Core c owns kv-head c and q-heads
4c..4c+3 (columns 256c:256c+256 of Wq, 64c:64c+64 of Wk/Wv, rows
256c:256c+256 of Wo). Each core computes its heads' attention for both
batches plus the partial o-projection y_c @ Wo_c; the host sums the 8
partials.

v3+ (vs v2 baseline, 531us -> ~395us measured):
  - scores row-packed: the two heads of an hp-pair run as concurrent
    row-tiled matmuls (tile rows 0:64 / 64:128) against a duplicated
    kTd [128, T]; one PE slot per key block instead of two (score matmul
    time halved on hw: 89us -> 46us).
  - rope rewritten: fp16 outputs, quarter-swaps read straight from PSUM
    (fewer + cheaper DVE ops), q kept as [128, 512] head-pairs.
  - softmax denominators: reciprocal_approx_fast (0.65us vs 4us DVE
    divide; must run at partition base 0 - base-64 slices miscompute) and
    the K=1 ones-broadcast matmuls (ran cold ~650ns) replaced by two K=2
    matmuls per chunk.
  - deeper software pipeline: projection emitted one chunk ahead of
    attention, normalize+o-projection two behind; attnV deferred 2 key
    blocks behind its exp so the in-order PE queue head never waits on
    Act; psum pools sized to exactly 8 banks (scores 2x2, y 2, proj 2).
  - warmup matmul burst on a memset tile fills the startup DMA window and
    holds the PE HAM clock-gate at 8/8.
  - engine rebalance: exp stays on Act (the per-block critical op);
    psum->sbuf copies split DVE/Act; causal corner masks on gpsimd; small
    latency-critical DMAs (rc2, transposes) on the sync queue - putting
    them on the scalar queue stalls exp dispatch (+48us, measured).

Rejected: fp8e4 DoubleRow for the projections. Attention output is a
weighted average, so the signal shrinks ~1/sqrt(n_eff) exactly as fast as
independent per-key quantization noise: each fp8 operand costs its full
~2.5% relative error on the output (measured 2-4% each, 5.4% combined vs
the 2e-2 gate), and DoubleRow's 256-col weight loads erased the matmul
gain anyway (335ns/MM measured vs 258ns bf16).
"""
import os

import numpy as np
import ml_dtypes

import concourse.mybir as mybir
import concourse.tile as tile
from concourse import bacc
from concourse.bass_utils import run_bass_kernel_spmd

F32 = mybir.dt.float32
BF16 = mybir.dt.bfloat16
FP16 = mybir.dt.float16
AF = mybir.ActivationFunctionType
BF16NP = ml_dtypes.bfloat16
FP16NP = np.float16

B, T, E = 2, 2048, 2048
H, HKV, D = 32, 8, 64
G = H // HKV          # q heads per kv head (= per core)
NCORES = 8
QH = G * D            # 256 q cols per core
ECH = E // 128        # 16 contraction chunks
TQC = 512             # tq chunk width
NTQ = T // TQC        # 4
NSB = T // 128        # 16 key blocks per batch
ROPE_BASE = 10000.0

_compiled = None
LAST_RESULT = None


def _build():
    nc = bacc.Bacc(None, target_bir_lowering=False, debug=False)

    NCH = B * T // TQC
    xt_d = nc.declare_dram_parameter("xT", [128, NCH, ECH, TQC], BF16, isOutput=False)
    wq_d = nc.declare_dram_parameter("wq", [128, ECH, QH], BF16, isOutput=False)
    wkv_d = nc.declare_dram_parameter("wkv", [128, ECH, 2 * D], BF16, isOutput=False)
    wo_d = nc.declare_dram_parameter("wo", [128, 2, E], BF16, isOutput=False)
    cos_d = nc.declare_dram_parameter("cos", [128, T], FP16, isOutput=False)
    sin_d = nc.declare_dram_parameter("sin", [128, T], FP16, isOutput=False)
    cm_d = nc.declare_dram_parameter("cmask", [128, 2, 128], BF16, isOutput=False)
    e2_d = nc.declare_dram_parameter("e2", [2, 128], BF16, isOutput=False)
    out_d = nc.declare_dram_parameter("out", [B * T, E], BF16, isOutput=True)

    with tile.TileContext(nc) as tc:
        with (
            tc.tile_pool(name="const", bufs=1) as cp,
            tc.tile_pool(name="acts", bufs=1) as ac,
            tc.tile_pool(name="xT", bufs=2) as xp,
            tc.tile_pool(name="work", bufs=2) as wp,
            tc.tile_pool(name="ps1", bufs=2, space="PSUM") as ps1,   # 2 banks
            tc.tile_pool(name="psS", bufs=2, space="PSUM") as psS,   # 4 banks
            tc.tile_pool(name="psY", bufs=2, space="PSUM") as psY,   # 2 banks
        ):
            # startup DMAs spread across queues so the first projection's
            # inputs (wq hp0 cols + xT chunk 0) land as early as possible.
            wq_sb = cp.tile([128, ECH, QH], BF16)
            for e0 in range(0, ECH, 4):
                nc.scalar.dma_start(wq_sb[:, e0:e0 + 4, 0:128],
                                    wq_d[:, e0:e0 + 4, 0:128])
            wkv_sb = cp.tile([128, ECH, 2 * D], BF16)
            nc.scalar.dma_start(wkv_sb[:], wkv_d[:])
            nc.scalar.dma_start(wq_sb[:, :, 128:QH], wq_d[:, :, 128:QH])
            cos_sb = cp.tile([128, T], FP16)
            nc.gpsimd.dma_start(cos_sb[:], cos_d[:, :])
            sin_sb = cp.tile([128, T], FP16)
            nc.gpsimd.dma_start(sin_sb[:], sin_d[:, :])
            cm_sb = cp.tile([128, 2, 128], BF16)
            nc.gpsimd.dma_start(cm_sb[:], cm_d[:])
            wo_sb = cp.tile([128, 2, E], BF16)
            nc.gpsimd.dma_start(wo_sb[:], wo_d[:])
            # E2[u, 64u:64u+64] = 1: K=2 stationary that broadcasts the two
            # per-group reciprocals of an hp-pair onto their 64-row blocks.
            e2_sb = cp.tile([2, 128], BF16)
            nc.gpsimd.dma_start(e2_sb[:], e2_d[:])

            # warmup: dummy matmuls on a memset tile fill the initial DMA
            # wait and push the PE HAM clock-gate to 8/8 before real work.
            wu_sb = cp.tile([128, TQC], BF16)
            nc.vector.memset(wu_sb[:], 0.0)
            wu_ps = ps1.tile([128, TQC], F32, tag="p1")
            for _ in range(12):
                nc.tensor.matmul(wu_ps[:], wu_sb[:, 0:128], wu_sb[:],
                                 start=True, stop=True)

            kTd, vS = {}, {}
            for b in range(B):
                # rope'd K duplicated on rows 64:128 so the pair-packed score
                # matmuls can row-tile (head g on array rows 0:63, head g+1
                # on 64:127, same key block stationary).
                kTd[b] = ac.tile([128, T], FP16, name=f"kTd{b}", tag=f"kTd{b}")
                for i in range(NSB):
                    vS[b, i] = ac.tile([128, D + 1], BF16, name=f"vS{b}_{i}",
                                       tag=f"vS{b}_{i}")
                    nc.vector.memset(vS[b, i][:, D:D + 1], 1.0)
            # unnormalized y (head-pair-major) and normalized y; column ranges
            # are disjoint across (b, j) so two shared tiles each suffice.
            yU, yN = {}, {}
            for hp in range(2):
                yU[hp] = ac.tile([128, B * T], BF16, name=f"yU{hp}", tag=f"yU{hp}")
                yN[hp] = ac.tile([128, B * T], BF16, name=f"yN{hp}", tag=f"yN{hp}")

            def rope128(dst, src, tc0, rows=128):
                # dst[0:rows, TQC] <- rope(psum_f32[0:rows, TQC]); fp16 out.
                # src must be PSUM: the quarter-swaps change base partition,
                # which SBUF-SBUF tensor_tensor cannot do.
                t1 = wp.tile([128, TQC], FP16, tag="t1")
                t2 = wp.tile([128, TQC], FP16, tag="t2")
                nc.vector.tensor_mul(t1[0:rows, :], src[0:rows, :],
                                     cos_sb[0:rows, tc0:tc0 + TQC])
                for q0 in range(0, rows, 64):
                    nc.vector.tensor_mul(t2[q0:q0 + 32, :],
                                         src[q0 + 32:q0 + 64, :],
                                         sin_sb[q0:q0 + 32, tc0:tc0 + TQC])
                    nc.vector.tensor_mul(t2[q0 + 32:q0 + 64, :],
                                         src[q0:q0 + 32, :],
                                         sin_sb[q0 + 32:q0 + 64, tc0:tc0 + TQC])
                nc.vector.tensor_add(dst, t1[0:rows, :], t2[0:rows, :])

            def emit_proj(b, j):
                rows0 = b * T + j * TQC
                tc0 = j * TQC
                ch = rows0 // TQC
                xT_t = xp.tile([128, ECH, TQC], BF16, tag="xT")
                nparts = 8 if (b, j) == (0, 0) else 1
                dq = nc.sync if ch % 2 == 0 else nc.scalar
                for q in range(nparts):
                    e0 = ECH // nparts * q
                    e1 = ECH // nparts * (q + 1)
                    dq.dma_start(xT_t[:, e0:e1, :], xt_d[:, ch, e0:e1, :])
                # q kept as [128, TQC] head-pairs: rows 0:64 head 2hp,
                # rows 64:128 head 2hp+1 -- exactly the psum layout.
                qP = [wp.tile([128, TQC], FP16, tag=f"qP{hp}", name=f"qP{hp}")
                      for hp in range(2)]
                for hp in range(2):
                    qp = ps1.tile([128, TQC], F32, tag="p1")
                    for ec in range(ECH):
                        nc.tensor.matmul(qp[:],
                                         wq_sb[:, ec, 128 * hp:128 * hp + 128],
                                         xT_t[:, ec, :],
                                         start=(ec == 0), stop=(ec == ECH - 1))
                    rope128(qP[hp][:, :], qp, tc0)
                kvp = ps1.tile([128, TQC], F32, tag="p1")
                for ec in range(ECH):
                    nc.tensor.matmul(kvp[:], wkv_sb[:, ec, :], xT_t[:, ec, :],
                                     start=(ec == 0), stop=(ec == ECH - 1))
                rope128(kTd[b][0:64, tc0:tc0 + TQC], kvp, tc0, rows=64)
                # duplicate the rope'd K onto rows 64:128 (row-tile partner).
                nc.scalar.dma_start(kTd[b][64:128, tc0:tc0 + TQC],
                                    kTd[b][0:64, tc0:tc0 + TQC])
                vT = wp.tile([64, TQC], BF16, tag="vT")
                for tb in range(4):
                    sl = slice(tb * 128, (tb + 1) * 128)
                    if tb % 2 == 0:
                        nc.scalar.copy(vT[:, sl], kvp[64:128, sl])
                    else:
                        nc.vector.tensor_copy(vT[:, sl], kvp[64:128, sl])
                    nc.sync.dma_start_transpose(
                        vS[b, j * 4 + tb][:, 0:D], vT[:, sl])
                return qP

            def emit_attn_core(b, j, qP):
                tc0 = j * TQC
                den = wp.tile([128, TQC], F32, tag="den")
                rcf = wp.tile([128, TQC], F32, tag="rcf")
                rcb = wp.tile([128, TQC], BF16, tag="rcb")
                rc2 = wp.tile([2, 2, TQC], BF16, tag="rc2")
                nsb = 4 * j + 4
                for hp in range(2):
                    yp = [psY.tile([65, TQC], F32, tag="y", name=f"yp{u}")
                          for u in range(2)]

                    def attnv(kb, j0, pt):
                        for u in range(2):
                            nc.tensor.matmul(yp[u][:, j0:TQC], vS[b, kb][:, :],
                                             pt[:, u, j0:TQC],
                                             start=(kb == 0), stop=(kb == nsb - 1),
                                             skip_group_check=True)

                    # attnV runs 2 key blocks behind its exp so it never
                    # stalls the in-order PE queue on the Act engine.
                    pend = []
                    for kb in range(nsb):
                        full = kb < 4 * j
                        j0 = 0 if full else 128 * (kb - 4 * j)
                        k0 = 128 * kb
                        sp = psS.tile([128, 2, TQC], F32, tag="s2")
                        pt = wp.tile([128, 2, TQC], BF16, tag="p2", bufs=3)
                        # row-packed pair: head 2hp on array rows 0:63,
                        # head 2hp+1 on rows 64:127, concurrent.
                        for u in range(2):
                            nc.tensor.matmul(sp[:, u, j0:TQC],
                                             kTd[b][64 * u:64 * u + 64, k0:k0 + 128],
                                             qP[hp][64 * u:64 * u + 64, j0:TQC],
                                             start=True, stop=True)
                        if full:
                            nc.scalar.activation(pt[:, :, :], sp[:, :, :],
                                                 AF.Exp, scale=0.125)
                        else:
                            nc.scalar.activation(pt[:, :, j0:TQC],
                                                 sp[:, :, j0:TQC],
                                                 AF.Exp, scale=0.125)
                            nc.gpsimd.tensor_mul(pt[:, :, j0:j0 + 128],
                                                 pt[:, :, j0:j0 + 128],
                                                 cm_sb[:])
                        pend.append((kb, j0, pt))
                        if len(pend) > 2:
                            attnv(*pend.pop(0))
                    while pend:
                        attnv(*pend.pop(0))
                    for u in range(2):
                        g = 2 * hp + u
                        nc.vector.tensor_copy(
                            yU[hp][64 * u:64 * u + 64, b * T + tc0:b * T + tc0 + TQC],
                            yp[u][0:64, :])
                        nc.vector.tensor_copy(den[32 * g:32 * g + 1, :],
                                              yp[u][64:65, :])
                with nc.allow_low_precision(reason="softmax denom bcast"):
                    nc.vector.reciprocal_approx_fast(rcf[:, :], den[:, :])
                    nc.vector.tensor_copy(rcb[:, :], rcf[:, :])
                for g in range(G):
                    nc.sync.dma_start(rc2[g % 2:g % 2 + 1, g // 2, :],
                                      rcb[32 * g:32 * g + 1, :])
                return rc2

            def emit_norm_oproj(b, j, rc2):
                tc0 = j * TQC
                c0 = b * T + tc0
                for hp in range(2):
                    bc = ps1.tile([128, TQC], F32, tag="p1")
                    nc.tensor.matmul(bc[:], e2_sb[:, :], rc2[:, hp, :],
                                     start=True, stop=True)
                    nc.vector.tensor_mul(yN[hp][:, c0:c0 + TQC],
                                         yU[hp][:, c0:c0 + TQC], bc[:])
                for tb in range(4):
                    r0 = tc0 + tb * 128
                    ot = wp.tile([128, E], BF16, tag="ot")
                    for ecol in range(4):
                        op = ps1.tile([128, TQC], F32, tag="p1")
                        for hc in range(2):
                            nc.tensor.matmul(op[:], yN[hc][:, b * T + r0:b * T + r0 + 128],
                                             wo_sb[:, hc, 512 * ecol:512 * ecol + 512],
                                             start=(hc == 0), stop=(hc == 1))
                        if ecol % 2 == 0:
                            nc.vector.tensor_copy(ot[:, 512 * ecol:512 * ecol + 512], op[:])
                        else:
                            nc.scalar.copy(ot[:, 512 * ecol:512 * ecol + 512], op[:])
                    nc.gpsimd.dma_start(out_d[b * T + r0:b * T + r0 + 128, :], ot[:])

            chunks = [(b, j) for b in range(B) for j in range(NTQ)]
            qPs, rcs = {}, {}
            for idx in range(len(chunks) + 2):
                if idx < len(chunks):
                    qPs[idx] = emit_proj(*chunks[idx])
                a = idx - 1
                if 0 <= a < len(chunks):
                    rcs[a] = emit_attn_core(*chunks[a], qPs.pop(a))
                n = idx - 2
                if 0 <= n < len(chunks):
                    emit_norm_oproj(*chunks[n], rcs.pop(n))

    nc.compile()
    return nc


def _host_consts():
    inv = ROPE_BASE ** (-np.arange(32, dtype=np.float64) / 32.0)
    ang = np.outer(inv, np.arange(T, dtype=np.float64))          # [32, T]
    cos128 = np.tile(np.cos(ang), (4, 1)).astype(FP16NP)         # [128, T]
    sin32 = np.sin(ang)
    sinS2 = np.concatenate([-sin32, sin32, -sin32, sin32], axis=0).astype(FP16NP)
    cmask = np.broadcast_to(np.triu(np.ones((128, 128)))[:, None, :],
                            (128, 2, 128)).copy().astype(BF16NP)  # valid iff p <= j
    e2 = np.zeros((2, 128), dtype=BF16NP)
    e2[0, 0:64] = 1
    e2[1, 64:128] = 1
    return cos128, sinS2, cmask, e2


def kernel(x, Wq, Wk, Wv, Wo):
    global _compiled, LAST_RESULT
    if _compiled is None:
        _compiled = _build()
    nc = _compiled

    xr = np.asarray(x, dtype=np.float32).reshape(B * T // TQC, TQC, ECH, 128)
    xtb = np.ascontiguousarray(xr.transpose(3, 0, 2, 1)).astype(BF16NP)
    cos128, sinS2, cmask, e2 = _host_consts()
    in_maps = []
    for c in range(NCORES):
        wkv = np.concatenate([Wk[:, D * c:D * (c + 1)], Wv[:, D * c:D * (c + 1)]],
                             axis=1)
        wqc = Wq[:, QH * c:QH * (c + 1)].reshape(ECH, 128, QH).transpose(1, 0, 2)
        wkvc = wkv.reshape(ECH, 128, 2 * D).transpose(1, 0, 2)
        woc = Wo[QH * c:QH * (c + 1), :].reshape(2, 128, E).transpose(1, 0, 2)
        in_maps.append({
            "xT": xtb,
            "wq": np.ascontiguousarray(wqc).astype(BF16NP),
            "wkv": np.ascontiguousarray(wkvc).astype(BF16NP),
            "wo": np.ascontiguousarray(woc).astype(BF16NP),
            "cos": cos128,
            "sin": sinS2,
            "cmask": cmask,
            "e2": e2,
        })
    trace = os.environ.get("GQA_TRACE", "0") == "1"
    res = run_bass_kernel_spmd(nc, in_maps, core_ids=list(range(NCORES)), trace=trace)
    LAST_RESULT = res
    acc = np.zeros((B * T, E), np.float32)
    for r in res.results:
        acc += np.asarray(r["out"]).astype(np.float32)
    return acc.reshape(B, T, E)


# revision 40
# speedup vs baseline: 1.1435x; 1.1435x over previous
"""GQA forward (B=2,T=2048,E=2048,H=32,HKV=8,D=64, RoPE, causal) on 8 trn2 cores.

Sharding: tensor-parallel over kv-heads. Core c owns kv-head c and q-heads
4c..4c+3 (columns 256c:256c+256 of Wq, 64c:64c+64 of Wk/Wv, rows
256c:256c+256 of Wo). Each core computes its heads' attention for both
batches plus the partial o-projection y_c @ Wo_c; the host sums the 8
partials.

v3 (vs v2 baseline, 531us):
  - scores row-packed: the two heads of an hp-pair run as concurrent
    row-tiled matmuls (tile rows 0:64 / 64:128) against a duplicated
    kTd [128, T]; one PE slot per key block instead of two.
  - rope rewritten: fp16 outputs, quarter-swaps read straight from PSUM
    (fewer + cheaper DVE ops), q kept as [128, 512] head-pairs (no per-head
    [64, x] tiles).
  - softmax denominators: reciprocal_approx_fast (~0.9us vs 4us DVE divide)
    and the K=1 ones-broadcast matmuls (ran cold at ~650ns) replaced by two
    K=2 matmuls per chunk.
  - deeper software pipeline: projection emitted one chunk ahead of
    attention, normalize+o-projection two behind; psum pools sized to
    exactly 8 banks (scores 2x2, y 2, proj 2).
  - engine rebalance: exp stays on Act (the per-block critical op);
    psum->sbuf copies split DVE/Act; causal corner masks on gpsimd.
"""
import os

import numpy as np
import ml_dtypes

import concourse.mybir as mybir
import concourse.tile as tile
from concourse import bacc
from concourse.bass_utils import run_bass_kernel_spmd

F32 = mybir.dt.float32
BF16 = mybir.dt.bfloat16
FP16 = mybir.dt.float16
AF = mybir.ActivationFunctionType
BF16NP = ml_dtypes.bfloat16
FP16NP = np.float16

B, T, E = 2, 2048, 2048
H, HKV, D = 32, 8, 64
G = H // HKV          # q heads per kv head (= per core)
NCORES = 8
QH = G * D            # 256 q cols per core
ECH = E // 128        # 16 contraction chunks
TQC = 512             # tq chunk width
NTQ = T // TQC        # 4
NSB = T // 128        # 16 key blocks per batch
ROPE_BASE = 10000.0

_compiled = None
LAST_RESULT = None


def _build():
    nc = bacc.Bacc(None, target_bir_lowering=False, debug=False)

    NCH = B * T // TQC
    xt_d = nc.declare_dram_parameter("xT", [128, NCH, ECH, TQC], BF16, isOutput=False)
    wq_d = nc.declare_dram_parameter("wq", [128, ECH, QH], BF16, isOutput=False)
    wkv_d = nc.declare_dram_parameter("wkv", [128, ECH, 2 * D], BF16, isOutput=False)
    wo_d = nc.declare_dram_parameter("wo", [128, 2, E], BF16, isOutput=False)
    cos_d = nc.declare_dram_parameter("cos", [128, T], FP16, isOutput=False)
    sin_d = nc.declare_dram_parameter("sin", [128, T], FP16, isOutput=False)
    cm_d = nc.declare_dram_parameter("cmask", [128, 128], BF16, isOutput=False)
    e2_d = nc.declare_dram_parameter("e2", [2, 128], BF16, isOutput=False)
    out_d = nc.declare_dram_parameter("out", [B * T, E], BF16, isOutput=True)

    with tile.TileContext(nc) as tc:
        with (
            tc.tile_pool(name="const", bufs=1) as cp,
            tc.tile_pool(name="acts", bufs=1) as ac,
            tc.tile_pool(name="xT", bufs=2) as xp,
            tc.tile_pool(name="work", bufs=2) as wp,
            tc.tile_pool(name="ps1", bufs=2, space="PSUM") as ps1,   # 2 banks
            tc.tile_pool(name="psS", bufs=2, space="PSUM") as psS,   # 4 banks
            tc.tile_pool(name="psY", bufs=2, space="PSUM") as psY,   # 2 banks
        ):
            # startup DMAs spread across queues so the first projection's
            # inputs (wq hp0 cols + xT chunk 0) land as early as possible.
            wq_sb = cp.tile([128, ECH, QH], BF16)
            for e0 in range(0, ECH, 4):
                nc.scalar.dma_start(wq_sb[:, e0:e0 + 4, 0:128],
                                    wq_d[:, e0:e0 + 4, 0:128])
            wkv_sb = cp.tile([128, ECH, 2 * D], BF16)
            nc.scalar.dma_start(wkv_sb[:], wkv_d[:])
            nc.scalar.dma_start(wq_sb[:, :, 128:QH], wq_d[:, :, 128:QH])
            cos_sb = cp.tile([128, T], FP16)
            nc.gpsimd.dma_start(cos_sb[:], cos_d[:, :])
            sin_sb = cp.tile([128, T], FP16)
            nc.gpsimd.dma_start(sin_sb[:], sin_d[:, :])
            cm_sb = cp.tile([128, 128], BF16)
            nc.gpsimd.dma_start(cm_sb[:], cm_d[:, :])
            wo_sb = cp.tile([128, 2, E], BF16)
            nc.gpsimd.dma_start(wo_sb[:], wo_d[:])
            # E2[u, 64u:64u+64] = 1: K=2 stationary that broadcasts the two
            # per-group reciprocals of an hp-pair onto their 64-row blocks.
            e2_sb = cp.tile([2, 128], BF16)
            nc.gpsimd.dma_start(e2_sb[:], e2_d[:])

            # warmup: dummy matmuls on a memset tile fill the initial DMA
            # wait and push the PE HAM clock-gate to 8/8 before real work.
            wu_sb = cp.tile([128, TQC], BF16)
            nc.vector.memset(wu_sb[:], 0.0)
            wu_ps = ps1.tile([128, TQC], F32, tag="p1")
            for _ in range(12):
                nc.tensor.matmul(wu_ps[:], wu_sb[:, 0:128], wu_sb[:],
                                 start=True, stop=True)

            kTd, vS = {}, {}
            for b in range(B):
                # rope'd K duplicated on rows 64:128 so the pair-packed score
                # matmuls can row-tile (head g on array rows 0:63, head g+1
                # on 64:127, same key block stationary).
                kTd[b] = ac.tile([128, T], FP16, name=f"kTd{b}", tag=f"kTd{b}")
                for i in range(NSB):
                    vS[b, i] = ac.tile([128, D + 1], BF16, name=f"vS{b}_{i}",
                                       tag=f"vS{b}_{i}")
                    nc.vector.memset(vS[b, i][:, D:D + 1], 1.0)
            # unnormalized y (head-pair-major) and normalized y; column ranges
            # are disjoint across (b, j) so two shared tiles each suffice.
            yU, yN = {}, {}
            for hp in range(2):
                yU[hp] = ac.tile([128, B * T], BF16, name=f"yU{hp}", tag=f"yU{hp}")
                yN[hp] = ac.tile([128, B * T], BF16, name=f"yN{hp}", tag=f"yN{hp}")

            def rope128(dst, src, tc0, rows=128):
                # dst[0:rows, TQC] <- rope(psum_f32[0:rows, TQC]); fp16 out.
                # src must be PSUM: the quarter-swaps change base partition,
                # which SBUF-SBUF tensor_tensor cannot do.
                t1 = wp.tile([128, TQC], FP16, tag="t1")
                t2 = wp.tile([128, TQC], FP16, tag="t2")
                nc.vector.tensor_mul(t1[0:rows, :], src[0:rows, :],
                                     cos_sb[0:rows, tc0:tc0 + TQC])
                for q0 in range(0, rows, 64):
                    nc.vector.tensor_mul(t2[q0:q0 + 32, :],
                                         src[q0 + 32:q0 + 64, :],
                                         sin_sb[q0:q0 + 32, tc0:tc0 + TQC])
                    nc.vector.tensor_mul(t2[q0 + 32:q0 + 64, :],
                                         src[q0:q0 + 32, :],
                                         sin_sb[q0 + 32:q0 + 64, tc0:tc0 + TQC])
                nc.vector.tensor_add(dst, t1[0:rows, :], t2[0:rows, :])

            def emit_proj(b, j):
                rows0 = b * T + j * TQC
                tc0 = j * TQC
                ch = rows0 // TQC
                xT_t = xp.tile([128, ECH, TQC], BF16, tag="xT")
                nparts = 8 if (b, j) == (0, 0) else 1
                dq = nc.sync if ch % 2 == 0 else nc.scalar
                for q in range(nparts):
                    e0 = ECH // nparts * q
                    e1 = ECH // nparts * (q + 1)
                    dq.dma_start(xT_t[:, e0:e1, :], xt_d[:, ch, e0:e1, :])
                # q kept as [128, TQC] head-pairs: rows 0:64 head 2hp,
                # rows 64:128 head 2hp+1 -- exactly the psum layout.
                qP = [wp.tile([128, TQC], FP16, tag=f"qP{hp}", name=f"qP{hp}")
                      for hp in range(2)]
                for hp in range(2):
                    qp = ps1.tile([128, TQC], F32, tag="p1")
                    for ec in range(ECH):
                        nc.tensor.matmul(qp[:],
                                         wq_sb[:, ec, 128 * hp:128 * hp + 128],
                                         xT_t[:, ec, :],
                                         start=(ec == 0), stop=(ec == ECH - 1))
                    rope128(qP[hp][:, :], qp, tc0)
                kvp = ps1.tile([128, TQC], F32, tag="p1")
                for ec in range(ECH):
                    nc.tensor.matmul(kvp[:], wkv_sb[:, ec, :], xT_t[:, ec, :],
                                     start=(ec == 0), stop=(ec == ECH - 1))
                rope128(kTd[b][0:64, tc0:tc0 + TQC], kvp, tc0, rows=64)
                # duplicate the rope'd K onto rows 64:128 (row-tile partner).
                nc.scalar.dma_start(kTd[b][64:128, tc0:tc0 + TQC],
                                    kTd[b][0:64, tc0:tc0 + TQC])
                vT = wp.tile([64, TQC], BF16, tag="vT")
                for tb in range(4):
                    sl = slice(tb * 128, (tb + 1) * 128)
                    if tb % 2 == 0:
                        nc.scalar.copy(vT[:, sl], kvp[64:128, sl])
                    else:
                        nc.vector.tensor_copy(vT[:, sl], kvp[64:128, sl])
                    nc.sync.dma_start_transpose(
                        vS[b, j * 4 + tb][:, 0:D], vT[:, sl])
                return qP

            def emit_attn_core(b, j, qP):
                tc0 = j * TQC
                den = wp.tile([128, TQC], F32, tag="den")
                rcf = wp.tile([128, TQC], F32, tag="rcf")
                rcb = wp.tile([128, TQC], BF16, tag="rcb")
                rc2 = wp.tile([2, 2, TQC], BF16, tag="rc2")
                nsb = 4 * j + 4
                for hp in range(2):
                    yp = [psY.tile([65, TQC], F32, tag="y", name=f"yp{u}")
                          for u in range(2)]

                    def attnv(kb, j0, pt):
                        for u in range(2):
                            nc.tensor.matmul(yp[u][:, j0:TQC], vS[b, kb][:, :],
                                             pt[:, u, j0:TQC],
                                             start=(kb == 0), stop=(kb == nsb - 1),
                                             skip_group_check=True)

                    # attnV runs 2 key blocks behind its exp so it never
                    # stalls the in-order PE queue on the Act engine.
                    pend = []
                    for kb in range(nsb):
                        full = kb < 4 * j
                        j0 = 0 if full else 128 * (kb - 4 * j)
                        k0 = 128 * kb
                        sp = psS.tile([128, 2, TQC], F32, tag="s2")
                        pt = wp.tile([128, 2, TQC], BF16, tag="p2", bufs=3)
                        # row-packed pair: head 2hp on array rows 0:63,
                        # head 2hp+1 on rows 64:127, concurrent.
                        for u in range(2):
                            nc.tensor.matmul(sp[:, u, j0:TQC],
                                             kTd[b][64 * u:64 * u + 64, k0:k0 + 128],
                                             qP[hp][64 * u:64 * u + 64, j0:TQC],
                                             start=True, stop=True)
                        if full:
                            nc.scalar.activation(pt[:, :, :], sp[:, :, :],
                                                 AF.Exp, scale=0.125)
                        else:
                            for u in range(2):
                                nc.scalar.activation(pt[:, u, j0:TQC],
                                                     sp[:, u, j0:TQC],
                                                     AF.Exp, scale=0.125)
                                nc.gpsimd.tensor_mul(pt[:, u, j0:j0 + 128],
                                                     pt[:, u, j0:j0 + 128],
                                                     cm_sb[:])
                        pend.append((kb, j0, pt))
                        if len(pend) > 2:
                            attnv(*pend.pop(0))
                    while pend:
                        attnv(*pend.pop(0))
                    for u in range(2):
                        g = 2 * hp + u
                        nc.vector.tensor_copy(
                            yU[hp][64 * u:64 * u + 64, b * T + tc0:b * T + tc0 + TQC],
                            yp[u][0:64, :])
                        nc.vector.tensor_copy(den[32 * g:32 * g + 1, :],
                                              yp[u][64:65, :])
                with nc.allow_low_precision(reason="softmax denom bcast"):
                    nc.vector.reciprocal_approx_fast(rcf[:, :], den[:, :])
                    nc.vector.tensor_copy(rcb[:, :], rcf[:, :])
                for g in range(G):
                    nc.sync.dma_start(rc2[g % 2:g % 2 + 1, g // 2, :],
                                      rcb[32 * g:32 * g + 1, :])
                return rc2

            def emit_norm_oproj(b, j, rc2):
                tc0 = j * TQC
                c0 = b * T + tc0
                for hp in range(2):
                    bc = ps1.tile([128, TQC], F32, tag="p1")
                    nc.tensor.matmul(bc[:], e2_sb[:, :], rc2[:, hp, :],
                                     start=True, stop=True)
                    nc.vector.tensor_mul(yN[hp][:, c0:c0 + TQC],
                                         yU[hp][:, c0:c0 + TQC], bc[:])
                for tb in range(4):
                    r0 = tc0 + tb * 128
                    ot = wp.tile([128, E], BF16, tag="ot")
                    for ecol in range(4):
                        op = ps1.tile([128, TQC], F32, tag="p1")
                        for hc in range(2):
                            nc.tensor.matmul(op[:], yN[hc][:, b * T + r0:b * T + r0 + 128],
                                             wo_sb[:, hc, 512 * ecol:512 * ecol + 512],
                                             start=(hc == 0), stop=(hc == 1))
                        if ecol % 2 == 0:
                            nc.vector.tensor_copy(ot[:, 512 * ecol:512 * ecol + 512], op[:])
                        else:
                            nc.scalar.copy(ot[:, 512 * ecol:512 * ecol + 512], op[:])
                    nc.gpsimd.dma_start(out_d[b * T + r0:b * T + r0 + 128, :], ot[:])

            chunks = [(b, j) for b in range(B) for j in range(NTQ)]
            qPs, rcs = {}, {}
            for idx in range(len(chunks) + 2):
                if idx < len(chunks):
                    qPs[idx] = emit_proj(*chunks[idx])
                a = idx - 1
                if 0 <= a < len(chunks):
                    rcs[a] = emit_attn_core(*chunks[a], qPs.pop(a))
                n = idx - 2
                if 0 <= n < len(chunks):
                    emit_norm_oproj(*chunks[n], rcs.pop(n))

    nc.compile()
    return nc


def _host_consts():
    inv = ROPE_BASE ** (-np.arange(32, dtype=np.float64) / 32.0)
    ang = np.outer(inv, np.arange(T, dtype=np.float64))          # [32, T]
    cos128 = np.tile(np.cos(ang), (4, 1)).astype(FP16NP)         # [128, T]
    sin32 = np.sin(ang)
    sinS2 = np.concatenate([-sin32, sin32, -sin32, sin32], axis=0).astype(FP16NP)
    cmask = np.triu(np.ones((128, 128))).astype(BF16NP)          # valid iff p <= j
    e2 = np.zeros((2, 128), dtype=BF16NP)
    e2[0, 0:64] = 1
    e2[1, 64:128] = 1
    return cos128, sinS2, cmask, e2


def kernel(x, Wq, Wk, Wv, Wo):
    global _compiled, LAST_RESULT
    if _compiled is None:
        _compiled = _build()
    nc = _compiled

    xr = np.asarray(x, dtype=np.float32).reshape(B * T // TQC, TQC, ECH, 128)
    xtb = np.ascontiguousarray(xr.transpose(3, 0, 2, 1)).astype(BF16NP)
    cos128, sinS2, cmask, e2 = _host_consts()
    in_maps = []
    for c in range(NCORES):
        wkv = np.concatenate([Wk[:, D * c:D * (c + 1)], Wv[:, D * c:D * (c + 1)]],
                             axis=1)
        wqc = Wq[:, QH * c:QH * (c + 1)].reshape(ECH, 128, QH).transpose(1, 0, 2)
        wkvc = wkv.reshape(ECH, 128, 2 * D).transpose(1, 0, 2)
        woc = Wo[QH * c:QH * (c + 1), :].reshape(2, 128, E).transpose(1, 0, 2)
        in_maps.append({
            "xT": xtb,
            "wq": np.ascontiguousarray(wqc).astype(BF16NP),
            "wkv": np.ascontiguousarray(wkvc).astype(BF16NP),
            "wo": np.ascontiguousarray(woc).astype(BF16NP),
            "cos": cos128,
            "sin": sinS2,
            "cmask": cmask,
            "e2": e2,
        })
    trace = os.environ.get("GQA_TRACE", "0") == "1"
    res = run_bass_kernel_spmd(nc, in_maps, core_ids=list(range(NCORES)), trace=trace)
    LAST_RESULT = res
    acc = np.zeros((B * T, E), np.float32)
    for r in res.results:
        acc += np.asarray(r["out"]).astype(np.float32)
    return acc.reshape(B, T, E)


# revision 44
# speedup vs baseline: 1.1609x; 1.0153x over previous
"""GQA forward (B=2,T=2048,E=2048,H=32,HKV=8,D=64, RoPE, causal) on 8 trn2 cores.

Sharding: tensor-parallel over kv-heads. Core c owns kv-head c and q-heads
4c..4c+3 (columns 256c:256c+256 of Wq, 64c:64c+64 of Wk/Wv, rows
256c:256c+256 of Wo). Each core computes its heads' attention for both
batches plus the partial o-projection y_c @ Wo_c; the host sums the 8
partials.

v3 (vs v2 baseline, 531us):
  - scores row-packed: the two heads of an hp-pair run as concurrent
    row-tiled matmuls (tile rows 0:64 / 64:128) against a duplicated
    kTd [128, T]; one PE slot per key block instead of two.
  - rope rewritten: fp16 outputs, quarter-swaps read straight from PSUM
    (fewer + cheaper DVE ops), q kept as [128, 512] head-pairs (no per-head
    [64, x] tiles).
  - softmax denominators: reciprocal_approx_fast (~0.9us vs 4us DVE divide)
    and the K=1 ones-broadcast matmuls (ran cold at ~650ns) replaced by two
    K=2 matmuls per chunk.
  - deeper software pipeline: projection emitted one chunk ahead of
    attention, normalize+o-projection two behind; psum pools sized to
    exactly 8 banks (scores 2x2, y 2, proj 2).
  - engine rebalance: exp stays on Act (the per-block critical op);
    psum->sbuf copies split DVE/Act; causal corner masks on gpsimd.
"""
import os

import numpy as np
import ml_dtypes

import concourse.mybir as mybir
import concourse.tile as tile
from concourse import bacc
from concourse.bass_utils import run_bass_kernel_spmd

F32 = mybir.dt.float32
BF16 = mybir.dt.bfloat16
FP16 = mybir.dt.float16
AF = mybir.ActivationFunctionType
BF16NP = ml_dtypes.bfloat16
FP16NP = np.float16

B, T, E = 2, 2048, 2048
H, HKV, D = 32, 8, 64
G = H // HKV          # q heads per kv head (= per core)
NCORES = 8
QH = G * D            # 256 q cols per core
ECH = E // 128        # 16 contraction chunks
TQC = 512             # tq chunk width
NTQ = T // TQC        # 4
NSB = T // 128        # 16 key blocks per batch
ROPE_BASE = 10000.0

_compiled = None
LAST_RESULT = None


def _build():
    nc = bacc.Bacc(None, target_bir_lowering=False, debug=False)

    NCH = B * T // TQC
    xt_d = nc.declare_dram_parameter("xT", [128, NCH, ECH, TQC], BF16, isOutput=False)
    wq_d = nc.declare_dram_parameter("wq", [128, ECH, QH], BF16, isOutput=False)
    wkv_d = nc.declare_dram_parameter("wkv", [128, ECH, 2 * D], BF16, isOutput=False)
    wo_d = nc.declare_dram_parameter("wo", [128, 2, E], BF16, isOutput=False)
    cos_d = nc.declare_dram_parameter("cos", [128, T], FP16, isOutput=False)
    sin_d = nc.declare_dram_parameter("sin", [128, T], FP16, isOutput=False)
    cm_d = nc.declare_dram_parameter("cmask", [128, 128], BF16, isOutput=False)
    e2_d = nc.declare_dram_parameter("e2", [2, 128], BF16, isOutput=False)
    out_d = nc.declare_dram_parameter("out", [B * T, E], BF16, isOutput=True)

    with tile.TileContext(nc) as tc:
        with (
            tc.tile_pool(name="const", bufs=1) as cp,
            tc.tile_pool(name="acts", bufs=1) as ac,
            tc.tile_pool(name="xT", bufs=3) as xp,
            tc.tile_pool(name="work", bufs=2) as wp,
            tc.tile_pool(name="ps1", bufs=2, space="PSUM") as ps1,   # 2 banks
            tc.tile_pool(name="psS", bufs=2, space="PSUM") as psS,   # 4 banks
            tc.tile_pool(name="psY", bufs=2, space="PSUM") as psY,   # 2 banks
        ):
            # startup DMAs spread across queues so the first projection's
            # inputs (wq hp0 cols + xT chunk 0) land as early as possible.
            wq_sb = cp.tile([128, ECH, QH], BF16)
            for e0 in range(0, ECH, 4):
                nc.scalar.dma_start(wq_sb[:, e0:e0 + 4, 0:128],
                                    wq_d[:, e0:e0 + 4, 0:128])
            wkv_sb = cp.tile([128, ECH, 2 * D], BF16)
            nc.scalar.dma_start(wkv_sb[:], wkv_d[:])
            nc.scalar.dma_start(wq_sb[:, :, 128:QH], wq_d[:, :, 128:QH])
            cos_sb = cp.tile([128, T], FP16)
            nc.gpsimd.dma_start(cos_sb[:], cos_d[:, :])
            sin_sb = cp.tile([128, T], FP16)
            nc.gpsimd.dma_start(sin_sb[:], sin_d[:, :])
            cm_sb = cp.tile([128, 128], BF16)
            nc.gpsimd.dma_start(cm_sb[:], cm_d[:, :])
            wo_sb = cp.tile([128, 2, E], BF16)
            nc.gpsimd.dma_start(wo_sb[:], wo_d[:])
            # E2[u, 64u:64u+64] = 1: K=2 stationary that broadcasts the two
            # per-group reciprocals of an hp-pair onto their 64-row blocks.
            e2_sb = cp.tile([2, 128], BF16)
            nc.gpsimd.dma_start(e2_sb[:], e2_d[:])

            # warmup: dummy matmuls on a memset tile fill the initial DMA
            # wait and push the PE HAM clock-gate to 8/8 before real work.
            wu_sb = cp.tile([128, TQC], BF16)
            nc.vector.memset(wu_sb[:], 0.0)
            wu_ps = ps1.tile([128, TQC], F32, tag="p1")
            for _ in range(20):
                nc.tensor.matmul(wu_ps[:], wu_sb[:, 0:128], wu_sb[:],
                                 start=True, stop=True)

            kTd, vS = {}, {}
            for b in range(B):
                # rope'd K duplicated on rows 64:128 so the pair-packed score
                # matmuls can row-tile (head g on array rows 0:63, head g+1
                # on 64:127, same key block stationary).
                kTd[b] = ac.tile([128, T], FP16, name=f"kTd{b}", tag=f"kTd{b}")
                for i in range(NSB):
                    vS[b, i] = ac.tile([128, D + 1], BF16, name=f"vS{b}_{i}",
                                       tag=f"vS{b}_{i}")
                    nc.vector.memset(vS[b, i][:, D:D + 1], 1.0)
            # unnormalized y (head-pair-major) and normalized y; column ranges
            # are disjoint across (b, j) so two shared tiles each suffice.
            yU, yN = {}, {}
            for hp in range(2):
                yU[hp] = ac.tile([128, B * T], BF16, name=f"yU{hp}", tag=f"yU{hp}")
                yN[hp] = ac.tile([128, B * T], BF16, name=f"yN{hp}", tag=f"yN{hp}")

            def rope128(dst, src, tc0, rows=128):
                # dst[0:rows, TQC] <- rope(psum_f32[0:rows, TQC]); fp16 out.
                # src must be PSUM: the quarter-swaps change base partition,
                # which SBUF-SBUF tensor_tensor cannot do.
                t1 = wp.tile([128, TQC], FP16, tag="t1")
                t2 = wp.tile([128, TQC], FP16, tag="t2")
                nc.vector.tensor_mul(t1[0:rows, :], src[0:rows, :],
                                     cos_sb[0:rows, tc0:tc0 + TQC])
                for q0 in range(0, rows, 64):
                    nc.vector.tensor_mul(t2[q0:q0 + 32, :],
                                         src[q0 + 32:q0 + 64, :],
                                         sin_sb[q0:q0 + 32, tc0:tc0 + TQC])
                    nc.vector.tensor_mul(t2[q0 + 32:q0 + 64, :],
                                         src[q0:q0 + 32, :],
                                         sin_sb[q0 + 32:q0 + 64, tc0:tc0 + TQC])
                nc.vector.tensor_add(dst, t1[0:rows, :], t2[0:rows, :])

            def emit_proj(b, j):
                rows0 = b * T + j * TQC
                tc0 = j * TQC
                ch = rows0 // TQC
                xT_t = xp.tile([128, ECH, TQC], BF16, tag="xT")
                nparts = 8 if (b, j) == (0, 0) else 1
                dq = nc.sync if ch % 2 == 0 else nc.scalar
                for q in range(nparts):
                    e0 = ECH // nparts * q
                    e1 = ECH // nparts * (q + 1)
                    dq.dma_start(xT_t[:, e0:e1, :], xt_d[:, ch, e0:e1, :])
                # q kept as [128, TQC] head-pairs: rows 0:64 head 2hp,
                # rows 64:128 head 2hp+1 -- exactly the psum layout.
                qP = [wp.tile([128, TQC], FP16, tag=f"qP{hp}", name=f"qP{hp}",
                              bufs=3)
                      for hp in range(2)]
                for hp in range(2):
                    qp = ps1.tile([128, TQC], F32, tag="p1")
                    for ec in range(ECH):
                        nc.tensor.matmul(qp[:],
                                         wq_sb[:, ec, 128 * hp:128 * hp + 128],
                                         xT_t[:, ec, :],
                                         start=(ec == 0), stop=(ec == ECH - 1))
                    rope128(qP[hp][:, :], qp, tc0)
                kvp = ps1.tile([128, TQC], F32, tag="p1")
                for ec in range(ECH):
                    nc.tensor.matmul(kvp[:], wkv_sb[:, ec, :], xT_t[:, ec, :],
                                     start=(ec == 0), stop=(ec == ECH - 1))
                rope128(kTd[b][0:64, tc0:tc0 + TQC], kvp, tc0, rows=64)
                # duplicate the rope'd K onto rows 64:128 (row-tile partner).
                nc.scalar.dma_start(kTd[b][64:128, tc0:tc0 + TQC],
                                    kTd[b][0:64, tc0:tc0 + TQC])
                vT = wp.tile([64, TQC], BF16, tag="vT")
                for tb in range(4):
                    sl = slice(tb * 128, (tb + 1) * 128)
                    if tb % 2 == 0:
                        nc.scalar.copy(vT[:, sl], kvp[64:128, sl])
                    else:
                        nc.vector.tensor_copy(vT[:, sl], kvp[64:128, sl])
                    nc.sync.dma_start_transpose(
                        vS[b, j * 4 + tb][:, 0:D], vT[:, sl])
                return qP

            def emit_attn_core(b, j, qP):
                tc0 = j * TQC
                den = wp.tile([128, TQC], F32, tag="den")
                rcf = wp.tile([128, TQC], F32, tag="rcf")
                rcb = wp.tile([128, TQC], BF16, tag="rcb")
                rc2 = wp.tile([2, 2, TQC], BF16, tag="rc2")
                nsb = 4 * j + 4
                for hp in range(2):
                    yp = [psY.tile([65, TQC], F32, tag="y", name=f"yp{u}")
                          for u in range(2)]

                    def attnv(kb, j0, pt):
                        for u in range(2):
                            nc.tensor.matmul(yp[u][:, j0:TQC], vS[b, kb][:, :],
                                             pt[:, u, j0:TQC],
                                             start=(kb == 0), stop=(kb == nsb - 1),
                                             skip_group_check=True)

                    # attnV runs 2 key blocks behind its exp so it never
                    # stalls the in-order PE queue on the Act engine.
                    pend = []
                    for kb in range(nsb):
                        full = kb < 4 * j
                        j0 = 0 if full else 128 * (kb - 4 * j)
                        k0 = 128 * kb
                        sp = psS.tile([128, 2, TQC], F32, tag="s2")
                        pt = wp.tile([128, 2, TQC], BF16, tag="p2", bufs=3)
                        # row-packed pair: head 2hp on array rows 0:63,
                        # head 2hp+1 on rows 64:127, concurrent.
                        for u in range(2):
                            nc.tensor.matmul(sp[:, u, j0:TQC],
                                             kTd[b][64 * u:64 * u + 64, k0:k0 + 128],
                                             qP[hp][64 * u:64 * u + 64, j0:TQC],
                                             start=True, stop=True)
                        if full:
                            nc.scalar.activation(pt[:, :, :], sp[:, :, :],
                                                 AF.Exp, scale=0.125)
                        else:
                            for u in range(2):
                                nc.scalar.activation(pt[:, u, j0:TQC],
                                                     sp[:, u, j0:TQC],
                                                     AF.Exp, scale=0.125)
                                nc.gpsimd.tensor_mul(pt[:, u, j0:j0 + 128],
                                                     pt[:, u, j0:j0 + 128],
                                                     cm_sb[:])
                        pend.append((kb, j0, pt))
                        if len(pend) > 2:
                            attnv(*pend.pop(0))
                    while pend:
                        attnv(*pend.pop(0))
                    for u in range(2):
                        g = 2 * hp + u
                        nc.vector.tensor_copy(
                            yU[hp][64 * u:64 * u + 64, b * T + tc0:b * T + tc0 + TQC],
                            yp[u][0:64, :])
                        nc.vector.tensor_copy(den[32 * g:32 * g + 1, :],
                                              yp[u][64:65, :])
                with nc.allow_low_precision(reason="softmax denom bcast"):
                    nc.vector.reciprocal_approx_fast(rcf[:, :], den[:, :])
                    nc.vector.tensor_copy(rcb[:, :], rcf[:, :])
                for g in range(G):
                    nc.sync.dma_start(rc2[g % 2:g % 2 + 1, g // 2, :],
                                      rcb[32 * g:32 * g + 1, :])
                return rc2

            def emit_norm_oproj(b, j, rc2):
                tc0 = j * TQC
                c0 = b * T + tc0
                for hp in range(2):
                    bc = ps1.tile([128, TQC], F32, tag="p1")
                    nc.tensor.matmul(bc[:], e2_sb[:, :], rc2[:, hp, :],
                                     start=True, stop=True)
                    nc.vector.tensor_mul(yN[hp][:, c0:c0 + TQC],
                                         yU[hp][:, c0:c0 + TQC], bc[:])
                for tb in range(4):
                    r0 = tc0 + tb * 128
                    ot = wp.tile([128, E], BF16, tag="ot")
                    for ecol in range(4):
                        op = ps1.tile([128, TQC], F32, tag="p1")
                        for hc in range(2):
                            nc.tensor.matmul(op[:], yN[hc][:, b * T + r0:b * T + r0 + 128],
                                             wo_sb[:, hc, 512 * ecol:512 * ecol + 512],
                                             start=(hc == 0), stop=(hc == 1))
                        if ecol % 2 == 0:
                            nc.vector.tensor_copy(ot[:, 512 * ecol:512 * ecol + 512], op[:])
                        else:
                            nc.scalar.copy(ot[:, 512 * ecol:512 * ecol + 512], op[:])
                    nc.gpsimd.dma_start(out_d[b * T + r0:b * T + r0 + 128, :], ot[:])

            # projection two chunks ahead of attention (fills the early
            # pipeline where attention is still small), normalize three
            # behind; qP/xT buffered one generation deeper to match.
            chunks = [(b, j) for b in range(B) for j in range(NTQ)]
            qPs, rcs = {}, {}
            for idx in range(len(chunks) + 3):
                if idx < len(chunks):
                    qPs[idx] = emit_proj(*chunks[idx])
                a = idx - 2
                if 0 <= a < len(chunks):
                    rcs[a] = emit_attn_core(*chunks[a], qPs.pop(a))
                n = idx - 3
                if 0 <= n < len(chunks):
                    emit_norm_oproj(*chunks[n], rcs.pop(n))

    nc.compile()
    return nc


def _host_consts():
    inv = ROPE_BASE ** (-np.arange(32, dtype=np.float64) / 32.0)
    ang = np.outer(inv, np.arange(T, dtype=np.float64))          # [32, T]
    cos128 = np.tile(np.cos(ang), (4, 1)).astype(FP16NP)         # [128, T]
    sin32 = np.sin(ang)
    sinS2 = np.concatenate([-sin32, sin32, -sin32, sin32], axis=0).astype(FP16NP)
    cmask = np.triu(np.ones((128, 128))).astype(BF16NP)          # valid iff p <= j
    e2 = np.zeros((2, 128), dtype=BF16NP)
    e2[0, 0:64] = 1
    e2[1, 64:128] = 1
    return cos128, sinS2, cmask, e2


def kernel(x, Wq, Wk, Wv, Wo):
    global _compiled, LAST_RESULT
    if _compiled is None:
        _compiled = _build()
    nc = _compiled

    xr = np.asarray(x, dtype=np.float32).reshape(B * T // TQC, TQC, ECH, 128)
    xtb = np.ascontiguousarray(xr.transpose(3, 0, 2, 1)).astype(BF16NP)
    cos128, sinS2, cmask, e2 = _host_consts()
    in_maps = []
    for c in range(NCORES):
        wkv = np.concatenate([Wk[:, D * c:D * (c + 1)], Wv[:, D * c:D * (c + 1)]],
                             axis=1)
        wqc = Wq[:, QH * c:QH * (c + 1)].reshape(ECH, 128, QH).transpose(1, 0, 2)
        wkvc = wkv.reshape(ECH, 128, 2 * D).transpose(1, 0, 2)
        woc = Wo[QH * c:QH * (c + 1), :].reshape(2, 128, E).transpose(1, 0, 2)
        in_maps.append({
            "xT": xtb,
            "wq": np.ascontiguousarray(wqc).astype(BF16NP),
            "wkv": np.ascontiguousarray(wkvc).astype(BF16NP),
            "wo": np.ascontiguousarray(woc).astype(BF16NP),
            "cos": cos128,
            "sin": sinS2,
            "cmask": cmask,
            "e2": e2,
        })
    trace = os.environ.get("GQA_TRACE", "0") == "1"
    res = run_bass_kernel_spmd(nc, in_maps, core_ids=list(range(NCORES)), trace=trace)
    LAST_RESULT = res
    acc = np.zeros((B * T, E), np.float32)
    for r in res.results:
        acc += np.asarray(r["out"]).astype(np.float32)
    return acc.reshape(B, T, E)
